# revision 1
# baseline (speedup 1.0000x reference)
"""Trainium2 Bass kernel for nn_MultiHeadAttention (B=4, S=2048, H=512, nh=4).

Sharding: 16 (batch, head) pairs over 8 cores -> each core computes one batch's
pair of heads (core = 2*b + head_pair). QKV projections are computed per-core
for just that core's 2 heads; attention runs in "St" orientation (scores
transposed, [k, q]) so that softmax'd weights feed the AV matmul with no
on-chip transposes:

  Qt[d,q] = relu((X W_q^T + b_q)/sqrt(dh))^T masked by (1-mask[q])
  St[k,q] = Kt^T. dot -> exp -> expSt (bf16)
  colsum[q] = ones^T @ expSt   (PE reduction over k)
  avT[d,q]  = V^T... = sum_k V[k,d] expSt[k,q]
  out[h*512 + 4d + c, r] = avT[d, c*512+r]/colsum + queries[...]   (model's
  faithful permute(0,1,3,2).reshape quirk folded into the output DMA pattern)

Masked queries: the row mask fills whole score rows with -1e9, so softmax is
uniform; we instead zero Qt's masked columns -> scores constant 0 -> exactly
uniform weights. All matmuls bf16 x bf16 with fp32 PSUM accumulation
(measured rel-l2 vs fp32 reference ~2e-4).
"""

import numpy as np
import ml_dtypes

import concourse.bacc as bacc
import concourse.bass as bass
import concourse.mybir as mybir
import concourse.tile as tile
from concourse.bass_utils import run_bass_kernel_spmd

B, S, H, NH, DH = 4, 2048, 512, 4, 128
N_CORES = 8
HC = H // 128          # contraction chunks for projections
KB = S // 128          # key blocks
F32 = mybir.dt.float32
BF16 = mybir.dt.bfloat16
BF = ml_dtypes.bfloat16
RELU = mybir.ActivationFunctionType.Relu
EXP = mybir.ActivationFunctionType.Exp
SQRT_DH = float(np.sqrt(DH))


def _emit(tc: "tile.TileContext", t) -> None:
    """Emit the per-core program. t is a dict of DRAM tensor handles."""
    nc = tc.nc

    with tc.tile_pool(name="consts", bufs=1) as consts, \
         tc.tile_pool(name="persist", bufs=1) as persist:
        # --- constants ---
        wq_sb = consts.tile([128, HC, 2 * DH], BF16, tag="wq")
        wk_sb = consts.tile([128, HC, 2 * DH], BF16, tag="wk")
        wv_sb = consts.tile([128, HC, 2 * DH], BF16, tag="wv")
        nc.sync.dma_start(out=wq_sb, in_=t["wq_t"].ap().rearrange("(c p) n -> p c n", p=128))
        nc.sync.dma_start(out=wk_sb, in_=t["wk_t"].ap().rearrange("(c p) n -> p c n", p=128))
        nc.sync.dma_start(out=wv_sb, in_=t["wv_t"].ap().rearrange("(c p) n -> p c n", p=128))
        bq_sb = consts.tile([128, 2], F32, tag="bq")
        bk_sb = consts.tile([128, 2], F32, tag="bk")
        nc.sync.dma_start(out=bq_sb, in_=t["bq"].ap().rearrange("(h p) -> p h", p=128))
        nc.sync.dma_start(out=bk_sb, in_=t["bk"].ap().rearrange("(h p) -> p h", p=128))
        bv_sb = consts.tile([1, 2 * DH], BF16, tag="bv")
        nc.sync.dma_start(out=bv_sb, in_=t["bv"].ap())
        ones_row = consts.tile([1, 128], BF16, tag="ones_row")
        ones_col = consts.tile([128, 1], BF16, tag="ones_col")
        nc.vector.memset(ones_row, 1.0)
        nc.vector.memset(ones_col, 1.0)
        # (1-mask) broadcast across partitions: [128, S] bf16
        fmask_bc = consts.tile([128, S], BF16, tag="fmask")
        fm = t["fmask"].ap()
        nc.gpsimd.dma_start(
            out=fmask_bc,
            in_=bass.AP(tensor=fm.tensor, offset=fm.offset, ap=[[0, 128], [1, S]]),
        )

        # --- persistent activations ---
        qtm_sb = persist.tile([128, 2, S], BF16, tag="qtm")   # masked Qt, 2 heads
        kt_sb = persist.tile([128, 2, S], BF16, tag="kt")
        v_sb = persist.tile([128, KB, 2 * DH], BF16, tag="v")  # V[k,d], s-major blocks

        # ================= projections =================
        with tc.tile_pool(name="xin", bufs=2) as xin_pool, \
             tc.tile_pool(name="proj_ps", bufs=2, space="PSUM") as proj_ps, \
             tc.tile_pool(name="vps", bufs=2, space="PSUM") as vps_pool, \
             tc.tile_pool(name="qtraw", bufs=2) as qtraw_pool:
            for ti in range(2):  # 0: Q, 1: K
                xt = t["xq_t"] if ti == 0 else t["xk_t"]
                w_sb = wq_sb if ti == 0 else wk_sb
                b_sb = bq_sb if ti == 0 else bk_sb
                scale = 1.0 / SQRT_DH if ti == 0 else 1.0
                xin = xin_pool.tile([128, HC, S], BF16, tag="xin")
                nc.sync.dma_start(out=xin, in_=xt.ap().rearrange("(c p) s -> p c s", p=128))
                for h in range(2):
                    for sc2 in range(2):  # 1024-wide output groups
                        ps = proj_ps.tile([128, 1024], F32, tag="pps")
                        for half in range(2):
                            s0 = (sc2 * 2 + half) * 512
                            for c in range(HC):
                                nc.tensor.matmul(
                                    ps[:, half * 512:(half + 1) * 512],
                                    lhsT=w_sb[:, c, h * DH:(h + 1) * DH],
                                    rhs=xin[:, c, s0:s0 + 512],
                                    start=(c == 0), stop=(c == HC - 1),
                                )
                        if ti == 1:
                            nc.scalar.activation(
                                out=kt_sb[:, h, sc2 * 1024:(sc2 + 1) * 1024], in_=ps,
                                func=RELU, bias=b_sb[:, h:h + 1], scale=scale,
                            )
                        else:
                            qr = qtraw_pool.tile([128, 1024], BF16, tag="qtraw")
                            nc.scalar.activation(
                                out=qr, in_=ps,
                                func=RELU, bias=b_sb[:, h:h + 1], scale=scale,
                            )
                            # mask out queries (whole-row mask quirk)
                            nc.vector.tensor_mul(
                                out=qtm_sb[:, h, sc2 * 1024:(sc2 + 1) * 1024],
                                in0=qr,
                                in1=fmask_bc[:, sc2 * 1024:(sc2 + 1) * 1024],
                            )
            # V projection: V[s, d] per 128-row block, bias via K=1 matmul
            xin_v = xin_pool.tile([128, HC, S], BF16, tag="xin")
            nc.sync.dma_start(out=xin_v, in_=t["xv_t"].ap().rearrange("(c p) s -> p c s", p=128))
            for sb in range(KB):
                vp = vps_pool.tile([128, 2 * DH], F32, tag="vps")
                for c in range(HC):
                    nc.tensor.matmul(
                        vp,
                        lhsT=xin_v[:, c, sb * 128:(sb + 1) * 128],
                        rhs=wv_sb[:, c, :],
                        start=(c == 0), stop=False,
                    )
                nc.tensor.matmul(vp, lhsT=ones_row, rhs=bv_sb, start=False, stop=True)
                nc.vector.tensor_scalar_max(out=v_sb[:, sb, :], in0=vp, scalar1=0.0)

        # ================= attention =================
        with tc.tile_pool(name="st_ps", bufs=2, space="PSUM") as st_pool, \
             tc.tile_pool(name="av_ps", bufs=1, space="PSUM") as av_pool, \
             tc.tile_pool(name="cs_ps", bufs=2, space="PSUM") as cs_pool, \
             tc.tile_pool(name="est", bufs=4) as est_pool, \
             tc.tile_pool(name="fin", bufs=2) as fin_pool, \
             tc.tile_pool(name="small", bufs=4) as small_pool:
            for h in range(2):
                for qc in range(2):  # 1024-wide query chunks
                    q0 = qc * 1024
                    av = av_pool.tile([128, 1024], F32, tag="av")
                    cs0 = cs_pool.tile([1, 512], F32, tag="cs")
                    cs1 = cs_pool.tile([1, 512], F32, tag="cs")
                    css = (cs0, cs1)
                    for g in range(KB):
                        st = st_pool.tile([128, 1024], F32, tag="st")
                        for half in range(2):
                            nc.tensor.matmul(
                                st[:, half * 512:(half + 1) * 512],
                                lhsT=kt_sb[:, h, g * 128:(g + 1) * 128],
                                rhs=qtm_sb[:, h, q0 + half * 512:q0 + (half + 1) * 512],
                                start=True, stop=True,
                            )
                        est = est_pool.tile([128, 1024], BF16, tag="est")
                        nc.scalar.activation(out=est, in_=st, func=EXP)
                        for half in range(2):
                            eh = est[:, half * 512:(half + 1) * 512]
                            nc.tensor.matmul(
                                css[half], lhsT=ones_col, rhs=eh,
                                start=(g == 0), stop=(g == KB - 1),
                            )
                            nc.tensor.matmul(
                                av[:, half * 512:(half + 1) * 512],
                                lhsT=v_sb[:, g, h * DH:(h + 1) * DH], rhs=eh,
                                start=(g == 0), stop=(g == KB - 1),
                            )
                    # normalization factors
                    csum = small_pool.tile([1, 1024], F32, tag="csum")
                    nc.vector.tensor_copy(out=csum[:, 0:512], in_=cs0)
                    nc.vector.tensor_copy(out=csum[:, 512:1024], in_=cs1)
                    recip = small_pool.tile([1, 1024], F32, tag="recip")
                    nc.vector.reciprocal_approx_fast(out=recip, in_=csum)
                    rb = fin_pool.tile([128, 1024], F32, tag="rb")
                    nc.gpsimd.partition_broadcast(rb, recip, channels=128)
                    # residual queries, permuted to match avT layout
                    resid_sb = fin_pool.tile([128, 1024], F32, tag="resid")
                    rs = t["resid"].ap()
                    for half in range(2):
                        c = qc * 2 + half
                        nc.sync.dma_start(
                            out=resid_sb[:, half * 512:(half + 1) * 512],
                            in_=bass.AP(
                                tensor=rs.tensor,
                                offset=rs.offset + (h * 512 + c) * H,
                                ap=[[4 * H, 128], [1, 512]],
                            ),
                        )
                    avn = fin_pool.tile([128, 1024], F32, tag="avn")
                    nc.vector.tensor_mul(out=avn, in0=rb, in1=av)
                    nc.vector.tensor_add(out=avn, in0=avn, in1=resid_sb)
                    ot = t["out"].ap()
                    for half in range(2):
                        c = qc * 2 + half
                        nc.sync.dma_start(
                            out=bass.AP(
                                tensor=ot.tensor,
                                offset=ot.offset + (h * 512 + c) * H,
                                ap=[[4 * H, 128], [1, 512]],
                            ),
                            in_=avn[:, half * 512:(half + 1) * 512],
                        )


def _build_nc():
    nc = bacc.Bacc("TRN2", target_bir_lowering=False, debug=False)
    t = {}
    t["xq_t"] = nc.dram_tensor("xq_t", [H, S], BF16, kind="ExternalInput")
    t["xk_t"] = nc.dram_tensor("xk_t", [H, S], BF16, kind="ExternalInput")
    t["xv_t"] = nc.dram_tensor("xv_t", [H, S], BF16, kind="ExternalInput")
    t["wq_t"] = nc.dram_tensor("wq_t", [H, 2 * DH], BF16, kind="ExternalInput")
    t["wk_t"] = nc.dram_tensor("wk_t", [H, 2 * DH], BF16, kind="ExternalInput")
    t["wv_t"] = nc.dram_tensor("wv_t", [H, 2 * DH], BF16, kind="ExternalInput")
    t["bq"] = nc.dram_tensor("bq", [2 * DH], F32, kind="ExternalInput")
    t["bk"] = nc.dram_tensor("bk", [2 * DH], F32, kind="ExternalInput")
    t["bv"] = nc.dram_tensor("bv", [1, 2 * DH], BF16, kind="ExternalInput")
    t["fmask"] = nc.dram_tensor("fmask", [S], BF16, kind="ExternalInput")
    t["resid"] = nc.dram_tensor("resid", [1024, H], F32, kind="ExternalInput")
    t["out"] = nc.dram_tensor("out", [1024, H], F32, kind="ExternalOutput")
    with tile.TileContext(nc) as tc:
        _emit(tc, t)
    nc.compile()
    return nc


_NC_CACHE = None


def _get_nc():
    global _NC_CACHE
    if _NC_CACHE is None:
        _NC_CACHE = _build_nc()
    return _NC_CACHE


def _core_inputs(core, queries, keys, values, attention_mask, Wq, bq, Wk, bk, Wv, bv):
    b = core // 2
    h0 = 2 * (core % 2)
    sl = slice(h0 * DH, (h0 + 2) * DH)
    return {
        "xq_t": np.ascontiguousarray(queries[b].T).astype(BF),
        "xk_t": np.ascontiguousarray(keys[b].T).astype(BF),
        "xv_t": np.ascontiguousarray(values[b].T).astype(BF),
        "wq_t": np.ascontiguousarray(Wq[sl, :].T).astype(BF),
        "wk_t": np.ascontiguousarray(Wk[sl, :].T).astype(BF),
        "wv_t": np.ascontiguousarray(Wv[sl, :].T).astype(BF),
        "bq": (bq[sl] / SQRT_DH).astype(np.float32),
        "bk": bk[sl].astype(np.float32),
        "bv": bv[sl].astype(BF).reshape(1, 2 * DH),
        "fmask": (1.0 - attention_mask[b].astype(np.float32)).astype(BF),
        "resid": np.ascontiguousarray(queries[b, h0 * 512:(h0 + 2) * 512, :]).astype(np.float32),
    }


def kernel(queries, keys, values, attention_mask, Wq, bq, Wk, bk, Wv, bv):
    queries = np.asarray(queries, dtype=np.float32)
    keys = np.asarray(keys, dtype=np.float32)
    values = np.asarray(values, dtype=np.float32)
    attention_mask = np.asarray(attention_mask)
    Wq, Wk, Wv = (np.asarray(a, dtype=np.float32) for a in (Wq, Wk, Wv))
    bq, bk, bv = (np.asarray(a, dtype=np.float32) for a in (bq, bk, bv))

    nc = _get_nc()
    in_maps = [
        _core_inputs(c, queries, keys, values, attention_mask, Wq, bq, Wk, bk, Wv, bv)
        for c in range(N_CORES)
    ]
    res = run_bass_kernel_spmd(nc, in_maps, core_ids=list(range(N_CORES)))
    out = np.empty((B, S, H), np.float32)
    for core in range(N_CORES):
        b = core // 2
        h0 = 2 * (core % 2)
        out[b, h0 * 512:(h0 + 2) * 512, :] = res.results[core]["out"]
    return out
